# revision 2
# baseline (speedup 1.0000x reference)
"""Trainium2 Bass kernel v2 for nn_AudioEvent.

Per-core pipeline (batch-parallel over 8 cores):
  osc:  host-precomputed quadratic phase coeffs (per 128-sample half-segment)
        -> stage2 matmuls (f32r, exact) -> mod 1.0 (DVE/Pool) -> Sin (Act,
        big tiles, arg 2*pi*frac - pi; env weights pre-negated)
        -> env-folded select matmuls (L/R interp nodes as 32 cols) -> *W blend
        -> pairsum matmul -> osc in (slot,e)-partition layout
  noise: host-transposed frames -> windowed rDFT matmuls -> gaussian filter
        (built on-chip) -> combined irfft+overlap-add matmuls directly into
        the (slot,e) layout
  mix:  piecewise-linear via per-partition node scalars + ramp consts
  out:  bf16, per-slot DMAs, host casts to f32
"""
import os
import numpy as np
import ml_dtypes

B, NE, NH, SEQ, N, WS = 8, 16, 32, 64, 16384, 512
NYQ = 11025.0
MIN_F0 = np.float64(20.0 / NYQ)
MAX_F0 = np.float64(800.0 / NYQ)
F0_DIFF = MAX_F0 - MIN_F0
NROW = NE * 33            # 528
NFR = SEQ * NE            # 1024
NSL = 64                  # 63 interior slots + head/tail combo slot

_cache = {}


def _build_static():
    if "static" in _cache:
        return _cache["static"]
    # ---- quadratic coeffs of V = cumsum(interp weights) ----
    pos = (np.arange(N, dtype=np.float64) + 0.5) * (SEQ / N) - 0.5
    pos = np.clip(pos, 0.0, SEQ - 1)
    i0 = np.floor(pos).astype(np.int64)
    i1 = np.minimum(i0 + 1, SEQ - 1)
    w = pos - i0
    U = np.zeros((SEQ, N))
    U[i0, np.arange(N)] += 1.0 - w
    U[i1, np.arange(N)] += w
    V = np.cumsum(U, axis=1)
    W64 = np.zeros((SEQ, 512))
    for m in range(32):
        for hs in range(4):
            t0 = 512 * m + 128 * hs
            A = V[:, t0]
            C = (V[:, t0 + 2] - 2 * V[:, t0 + 1] + V[:, t0]) / 2
            Bc = V[:, t0 + 1] - V[:, t0] - C
            W64[:, 16 * m + 3 * hs + 0] = A
            W64[:, 16 * m + 3 * hs + 1] = Bc
            W64[:, 16 * m + 3 * hs + 2] = C

    # ---- stage2 basis: even/odd paired [32, 512] ----
    j = np.arange(128, dtype=np.float64)
    basis16 = np.zeros((16, 512), np.float32)
    for hs in range(4):
        basis16[3 * hs + 0, 128 * hs:128 * (hs + 1)] = 1.0
        basis16[3 * hs + 1, 128 * hs:128 * (hs + 1)] = j
        basis16[3 * hs + 2, 128 * hs:128 * (hs + 1)] = j * j
    basisE = np.zeros((128, 512), np.float32)
    basisO = np.zeros((128, 512), np.float32)
    for a in range(4):
        basisE[32 * a:32 * a + 16] = basis16
        basisO[32 * a + 16:32 * a + 32] = basis16
    # fp16 split basis: 16 logical rows -> (1, j, j2hi, j2lo) x 4 half-segs,
    # each logical row replicated at 3 split slots (4th slot zero pad);
    # row = 64*rep + 4*br + s, two 64-row replicas
    b16s = np.zeros((16, 512))
    for hs in range(4):
        sl = slice(128 * hs, 128 * (hs + 1))
        b16s[4 * hs + 0, sl] = 1.0
        b16s[4 * hs + 1, sl] = j
        j2h = np.float16(j * j).astype(np.float64)
        b16s[4 * hs + 2, sl] = j2h
        b16s[4 * hs + 3, sl] = j * j - j2h
    basis64 = np.zeros((128, 512), np.float16)
    for rep in range(2):
        for br in range(16):
            for sp in range(3):
                basis64[64 * rep + 4 * br + sp] = np.float16(b16s[br])

    # ---- DFT consts (win folded), 4 contraction groups ----
    t = np.arange(WS)
    f = np.arange(WS // 2 + 1)
    win = 0.5 - 0.5 * np.cos(2 * np.pi * t / WS)
    ang = 2 * np.pi * np.outer(t, f) / WS
    CwRe = np.cos(ang) * win[:, None]
    CwIm = -np.sin(ang) * win[:, None]
    wgt = np.full(WS // 2 + 1, 2.0)
    wgt[0] = 1.0
    wgt[-1] = 1.0
    ang2 = 2 * np.pi * np.outer(f, t) / WS
    DRe = wgt[:, None] * np.cos(ang2) / WS
    DIm = -wgt[:, None] * np.sin(ang2) / WS
    # groups: 0: re f0..127 | 1: re f128..255 | 2: im f1..128 | 3: im f129..255 + re256
    Cw = np.zeros((128, 2048))       # col = tc*512 + grp*128 + fcol
    Dc = np.zeros((128, 2048))       # col = grp*512 + tau
    freq4 = np.zeros((128, 4))
    for grp in range(4):
        if grp == 0:
            fidx, mats = np.arange(0, 128), CwRe
        elif grp == 1:
            fidx, mats = np.arange(128, 256), CwRe
        elif grp == 2:
            fidx, mats = np.arange(1, 129), CwIm
        else:
            fidx, mats = np.concatenate([np.arange(129, 256), [256]]), None
        for tc in range(4):
            trows = slice(128 * tc, 128 * (tc + 1))
            if grp < 3:
                Cw[:, tc * 512 + grp * 128: tc * 512 + grp * 128 + 128] = mats[trows][:, fidx]
            else:
                blockm = CwIm[trows][:, fidx[:-1]]
                Cw[:, tc * 512 + grp * 128: tc * 512 + grp * 128 + 127] = blockm
                Cw[:, tc * 512 + grp * 128 + 127] = CwRe[trows][:, 256]
        if grp < 3:
            Dc[:, grp * 512:(grp + 1) * 512] = (DRe if grp < 2 else DIm)[fidx]
            freq4[:, grp] = fidx / 256.0
        else:
            Dc[:127, grp * 512:(grp + 1) * 512] = DIm[fidx[:-1]]
            Dc[127, grp * 512:(grp + 1) * 512] = DRe[256]
            freq4[:127, grp] = fidx[:-1] / 256.0
            freq4[127, grp] = 1.0

    # ---- ramp consts ----
    wj = (np.arange(256) + 0.5) / 256.0
    Wc = np.zeros((128, 256))
    for p in range(128):
        Wc[p] = wj if (p // 16) % 2 == 1 else 1.0 - wj
    Wc63 = Wc.copy()
    Wc63[96:112] = np.concatenate([np.ones(128), np.zeros(128)])
    Wc63[112:128] = np.concatenate([np.zeros(128), np.ones(128)])
    W0m = np.tile(1.0 - wj, (128, 1))
    W1m = np.tile(wj, (128, 1))
    W0m63 = W0m.copy()
    W1m63 = W1m.copy()
    W0m63[112:128] = np.concatenate([np.ones(128), np.zeros(128)])
    W1m63[112:128] = np.concatenate([np.zeros(128), np.ones(128)])
    P = np.zeros((128, 64))
    for p in range(128):
        P[p, 16 * (p // 32) + p % 16] = 1.0

    bf = ml_dtypes.bfloat16
    static = dict(
        W64=W64, basis64=basis64, b16s=b16s,
        Cw=Cw.astype(bf), Dc=Dc.astype(bf), freq4=freq4.astype(np.float32),
        Wc=Wc.astype(bf), Wc63=Wc63.astype(bf),
        W0m=W0m.astype(bf), W1m=W1m.astype(bf),
        W0m63=W0m63.astype(bf), W1m63=W1m63.astype(bf),
        P=P.astype(bf),
    )
    _cache["static"] = static
    return static


def _build_nc():
    if "nc" in _cache:
        return _cache["nc"]
    from concourse import bacc
    import concourse.tile as tile
    from concourse import mybir
    from contextlib import ExitStack

    F32 = mybir.dt.float32
    F16 = mybir.dt.float16
    BF16 = mybir.dt.bfloat16
    AF = mybir.ActivationFunctionType
    OP = mybir.AluOpType
    PI = float(np.pi)

    nc = bacc.Bacc()
    # data params
    coefT7 = nc.declare_dram_parameter("coefT7", [128, 5 * 2048], F16, isOutput=False)
    selW = nc.declare_dram_parameter("selW", [128, 8 * 2048], BF16, isOutput=False)
    ovn = nc.declare_dram_parameter("ovn", [128, 16], F32, isOutput=False)
    nfT = nc.declare_dram_parameter("nfT", [512, 1024], BF16, isOutput=False)
    meanb = nc.declare_dram_parameter("meanb", [128, 1056], BF16, isOutput=False)
    c2b = nc.declare_dram_parameter("c2b", [128, 1056], BF16, isOutput=False)
    # const params
    basis64 = nc.declare_dram_parameter("basis64", [128, 512], F16, isOutput=False)
    coefT4s = nc.declare_dram_parameter("coefT4s", [128, 1024], F16, isOutput=False)
    Cw = nc.declare_dram_parameter("Cw", [128, 2048], BF16, isOutput=False)
    Dc = nc.declare_dram_parameter("Dc", [128, 2048], BF16, isOutput=False)
    freq4 = nc.declare_dram_parameter("freq4", [128, 4], F32, isOutput=False)
    Wc = nc.declare_dram_parameter("Wc", [128, 256], BF16, isOutput=False)
    Wc63 = nc.declare_dram_parameter("Wc63", [128, 256], BF16, isOutput=False)
    W0m = nc.declare_dram_parameter("W0m", [128, 256], BF16, isOutput=False)
    W1m = nc.declare_dram_parameter("W1m", [128, 256], BF16, isOutput=False)
    W0m63 = nc.declare_dram_parameter("W0m63", [128, 256], BF16, isOutput=False)
    W1m63 = nc.declare_dram_parameter("W1m63", [128, 256], BF16, isOutput=False)
    P = nc.declare_dram_parameter("P", [128, 64], BF16, isOutput=False)
    out = nc.declare_dram_parameter("out", [NE, N], BF16, isOutput=True)

    with tile.TileContext(nc) as tc, ExitStack() as ctx:
        cp = ctx.enter_context(tc.tile_pool(name="const", bufs=1))
        # noise consts first (phase A starts immediately)
        nfT_sb = [cp.tile([128, 1024], BF16, tag=f"nfT{i}", name=f"nfT{i}") for i in range(4)]
        for i in range(4):
            nc.sync.dma_start(nfT_sb[i][:], nfT[128 * i:128 * (i + 1), :])
        meanb_sb = cp.tile([128, 1056], BF16, tag="meanb")
        nc.sync.dma_start(meanb_sb[:], meanb[:])
        c2b_sb = cp.tile([128, 1056], BF16, tag="c2b")
        nc.sync.dma_start(c2b_sb[:], c2b[:])
        Cw_sb = cp.tile([128, 2048], BF16, tag="Cw")
        nc.sync.dma_start(Cw_sb[:], Cw[:])
        freq4_sb = cp.tile([128, 4], F32, tag="freq4")
        nc.sync.dma_start(freq4_sb[:], freq4[:])
        Dc_sb = cp.tile([128, 2048], BF16, tag="Dc")
        nc.sync.dma_start(Dc_sb[:], Dc[:])
        # phase B consts (queued behind; SP has slack)
        coefT7_sb = cp.tile([128, 5 * 2048], F16, tag="coefT7")
        for bb in range(5):
            nc.sync.dma_start(coefT7_sb[:, 2048 * bb:2048 * (bb + 1)],
                              coefT7[:, 2048 * bb:2048 * (bb + 1)])
        basis64_sb = cp.tile([128, 512], F16, tag="basis64")
        nc.sync.dma_start(basis64_sb[:], basis64[:])
        coefT4s_sb = cp.tile([128, 1024], F16, tag="coefT4s")
        nc.sync.dma_start(coefT4s_sb[:], coefT4s[:])
        selW_sb = cp.tile([128, 8 * 2048], BF16, tag="selW")
        for bb in range(8):
            nc.sync.dma_start(selW_sb[:, 2048 * bb:2048 * (bb + 1)],
                              selW[:, 2048 * bb:2048 * (bb + 1)])
        P_sb = cp.tile([128, 64], BF16, tag="P")
        nc.sync.dma_start(P_sb[:], P[:])
        Wc_sb = cp.tile([128, 256], BF16, tag="Wc")
        nc.sync.dma_start(Wc_sb[:], Wc[:])
        Wc63_sb = cp.tile([128, 256], BF16, tag="Wc63")
        nc.sync.dma_start(Wc63_sb[:], Wc63[:])
        W0m_sb = cp.tile([128, 256], BF16, tag="W0m")
        nc.sync.dma_start(W0m_sb[:], W0m[:])
        W1m_sb = cp.tile([128, 256], BF16, tag="W1m")
        nc.sync.dma_start(W1m_sb[:], W1m[:])
        W0m63_sb = cp.tile([128, 256], BF16, tag="W0m63")
        nc.sync.dma_start(W0m63_sb[:], W0m63[:])
        W1m63_sb = cp.tile([128, 256], BF16, tag="W1m63")
        nc.sync.dma_start(W1m63_sb[:], W1m63[:])
        ovn_sb = cp.tile([128, 16], F32, tag="ovn")
        nc.sync.dma_start(ovn_sb[:], ovn[:])
        c23b = cp.tile([128, 1], F32, tag="c23b")
        nc.vector.memset(c23b[:], float(2.0 ** 23))

        spec_sb = [cp.tile([128, 1056], BF16, tag=f"spec{g}", name=f"spec{g}") for g in range(4)]
        noise_sb = [cp.tile([128, 256], BF16, tag=f"nz{t}", name=f"nz{t}") for t in range(8)]

        # ============ Phase A: noise ============
        with ExitStack() as actx:
            fpool = actx.enter_context(tc.tile_pool(name="fp", bufs=2))
            psA = actx.enter_context(tc.tile_pool(name="psA", bufs=2, space="PSUM"))
            psNZ = actx.enter_context(tc.tile_pool(name="psNZ", bufs=2, space="PSUM"))
            for g in range(4):
                nc.gpsimd.memset(spec_sb[g][:, 0:16], 0.0)
                nc.gpsimd.memset(spec_sb[g][:, 1040:1056], 0.0)
            # build filter then immediately rfft+mult for each group
            for g in range(4):
                d = fpool.tile([128, 1056], BF16, tag="fd")
                nc.gpsimd.tensor_scalar(d[:], meanb_sb[:], freq4_sb[:, g:g + 1], None, OP.subtract)
                d2 = fpool.tile([128, 1056], BF16, tag="fd2")
                nc.gpsimd.tensor_tensor(d2[:], d[:], d[:], OP.mult)
                m2 = fpool.tile([128, 1056], BF16, tag="fm2")
                nc.gpsimd.tensor_tensor(m2[:], d2[:], c2b_sb[:], OP.mult)
                filt = fpool.tile([128, 1056], BF16, tag=f"filt{g}")
                nc.scalar.activation(filt[:], m2[:], AF.Exp)
                for h in range(2):
                    fr_sl = slice(512 * h, 512 * (h + 1))
                    sp = psA.tile([128, 512], F32, tag="rf")
                    for tcx in range(4):
                        nc.tensor.matmul(sp[:],
                                         Cw_sb[:, tcx * 512 + g * 128: tcx * 512 + g * 128 + 128],
                                         nfT_sb[tcx][:, fr_sl],
                                         start=(tcx == 0), stop=(tcx == 3))
                    srf = fpool.tile([128, 512], BF16, tag="srf")
                    nc.scalar.copy(srf[:], sp[:])
                    nc.gpsimd.tensor_tensor(spec_sb[g][:, 16 + 512 * h:16 + 512 * (h + 1)],
                                            srf[:], filt[:, 16 + 512 * h:16 + 512 * (h + 1)],
                                            OP.mult)
            # irfft + overlap-add into (slot, e) layout
            for t in range(8):
                pz = psNZ.tile([128, 256], F32, tag="nzps")
                base = 16 + 16 * (8 * t)          # spec col of fr(slot=8t)
                nslots = 7 if t == 7 else 8
                ncols = 16 * nslots
                first = True
                for g in range(4):
                    gD = Dc_sb[:, 512 * g: 512 * (g + 1)]
                    sW = spec_sb[g]
                    # A: y[fr(s), 128+j] -> [:, 0:128]
                    nc.tensor.matmul(pz[0:ncols, 0:128], sW[:, base:base + ncols],
                                     gD[:, 128:256], start=first, stop=False,
                                     skip_group_check=True)
                    first = False
                    # C: y[fr(s-1), 384+j] -> [:, 0:128]
                    nc.tensor.matmul(pz[0:ncols, 0:128], sW[:, base - 16:base - 16 + ncols],
                                     gD[:, 384:512], start=False, stop=False,
                                     skip_group_check=True)
                    # B: y[fr(s+1), j] -> [:, 128:256]
                    nc.tensor.matmul(pz[0:ncols, 128:256], sW[:, base + 16:base + 16 + ncols],
                                     gD[:, 0:128], start=False, stop=False,
                                     skip_group_check=True)
                    # D: y[fr(s), 256+j] -> [:, 128:256]
                    nc.tensor.matmul(pz[0:ncols, 128:256], sW[:, base:base + ncols],
                                     gD[:, 256:384], start=False,
                                     stop=(t < 7 and g == 3),
                                     skip_group_check=True)
                if t == 7:
                    b63 = 16 + 16 * 63
                    pz63 = psNZ.tile([32, 256], F32, tag="nz63")
                    for g in range(4):
                        gD = Dc_sb[:, 512 * g: 512 * (g + 1)]
                        sW = spec_sb[g]
                        # head: y[fr(0), j] (+ zero-pad tail term via pad cols)
                        nc.tensor.matmul(pz63[0:16, 0:128], sW[:, 16:32],
                                         gD[:, 0:128], start=(g == 0), stop=False,
                                         skip_group_check=True, tile_position=(0, 0))
                        # tail: y[fr(63), 128+j] + y[fr(62), 384+j]
                        nc.tensor.matmul(pz63[0:16, 128:256], sW[:, b63:b63 + 16],
                                         gD[:, 128:256], start=False, stop=False,
                                         skip_group_check=True, tile_position=(0, 0))
                        last = (g == 3)
                        nc.tensor.matmul(pz63[0:16, 128:256], sW[:, b63 - 16:b63],
                                         gD[:, 384:512], start=False, stop=last,
                                         skip_group_check=True, tile_position=(0, 0))
                    nc.scalar.copy(noise_sb[t][0:112, :], pz[0:112, :])
                    nz63s = fpool.tile([16, 256], BF16, tag="nz63s")
                    nc.scalar.copy(nz63s[:], pz63[0:16, :])
                    nc.sync.dma_start(noise_sb[t][112:128, :], nz63s[:])
                else:
                    nc.scalar.copy(noise_sb[t][:], pz[:])

        # ============ Phase B: oscillator bank ============
        stp = ctx.enter_context(tc.tile_pool(name="st", bufs=2))
        st4p = ctx.enter_context(tc.tile_pool(name="st4", bufs=2))
        php = ctx.enter_context(tc.tile_pool(name="phi", bufs=2))
        ph4p = ctx.enter_context(tc.tile_pool(name="phi4", bufs=2))
        awp = ctx.enter_context(tc.tile_pool(name="aw", bufs=2))
        rtp = ctx.enter_context(tc.tile_pool(name="rt", bufs=2))
        fin = ctx.enter_context(tc.tile_pool(name="fin", bufs=2))
        psPH = ctx.enter_context(tc.tile_pool(name="psPH", bufs=2, space="PSUM"))
        psP4 = ctx.enter_context(tc.tile_pool(name="psP4", bufs=1, space="PSUM"))
        psSel = ctx.enter_context(tc.tile_pool(name="psSel", bufs=2, space="PSUM"))
        psSel3 = ctx.enter_context(tc.tile_pool(name="psSel3", bufs=2, space="PSUM"))
        psOsc = ctx.enter_context(tc.tile_pool(name="psOsc", bufs=1, space="PSUM"))

        st_tiles = {}   # (b, m) -> tile ; kept for current & previous span
        st4_tiles = {}
        mod_ctr = [0]

        C23 = float(2.0 ** 23)

        def do_mod(dst_ap, src_ap):
            """dst = frac-reduced phase in [-0.5, 0.5] via +-2^23 round trick.
            Rotates across engine assignments to balance load."""
            r = mod_ctr[0] % 4
            mod_ctr[0] += 1
            n_p = src_ap.partition_size()
            n_f = src_ap.free_size()
            yt = rtp.tile([128, 512], F32, tag="yt", name="yt")
            if r < 3:
                # X: Act yt (psum+2^23 -> sbuf, rounds at f32 write)
                nc.scalar.activation(yt[0:n_p, 0:n_f], src_ap, AF.Identity,
                                     bias=c23b[0:n_p, 0:1])
            else:
                # W: DVE yt
                nc.vector.tensor_scalar(yt[0:n_p, 0:n_f], src_ap,
                                        C23, None, OP.add)
            kt = rtp.tile([128, 512], F32, tag="kt", name="kt")
            nc.gpsimd.tensor_scalar(kt[0:n_p, 0:n_f], yt[0:n_p, 0:n_f],
                                    C23, None, OP.subtract)
            nc.vector.tensor_tensor(dst_ap, src_ap, kt[0:n_p, 0:n_f], OP.subtract)

        def emit_span(m):
            """stage2 + mod + sin for span m (samples 2048m..2048m+2047)."""
            for b in range(4):
                phi = php.tile([128, 2048], F32, tag="phi", name=f"phi{b}")
                for k in range(4):
                    mm = 4 * m + k
                    p2 = 64 * (mm % 2)
                    cb = 2048 * b + (mm // 2) * 128
                    pp = psPH.tile([128, 512], F32, tag="ph")
                    nc.tensor.matmul(pp[:],
                                     coefT7_sb[p2:p2 + 64, cb:cb + 128],
                                     basis64_sb[p2:p2 + 64, :], start=True, stop=True,
                                     skip_group_check=True, tile_position=(p2, 0))
                    do_mod(phi[:, 512 * k:512 * (k + 1)], pp[:])
                st = stp.tile([128, 2048], BF16, tag=f"st{b}", name=f"st{b}_{m}")
                nc.scalar.activation(st[:], phi[:], AF.Sin, scale=2 * PI)
                st_tiles[(b, m)] = st
            # block 4 (g 512..527), 4 chunks packed at 32-aligned bases
            phi4 = ph4p.tile([128, 512], F32, tag="phi4")
            pp4 = psP4.tile([128, 512], F32, tag="ph4")
            for kp in range(2):
                nc.tensor.matmul(pp4[64 * kp:64 * kp + 64, :],
                                 coefT4s_sb[:, 64 * (2 * m + kp):64 * (2 * m + kp) + 64],
                                 basis64_sb[:], start=True, stop=True,
                                 skip_group_check=True, tile_position=(0, 64 * kp))
            do_mod(phi4[:], pp4[:])
            st4 = st4p.tile([128, 512], BF16, tag="st4")
            nc.scalar.activation(st4[:], phi4[:], AF.Sin, scale=2 * PI)
            st4_tiles[m] = st4

        def sel_windows(slot):
            """windows into st spans for slot; returns list of (m, lo, hi, psum_lo)."""
            t0 = 128 + 256 * slot
            m = t0 // 2048
            lo = t0 - 2048 * m
            if lo + 256 <= 2048:
                return [(m, lo, lo + 256, 0)]
            return [(m, lo, 2048, 0), (m + 1, 0, lo + 256 - 2048, 2048 - lo)]

        def sel4_windows(slot):
            """block4 windows: (span, chunk-in-span, lo, hi, psum_lo)."""
            t0 = 128 + 256 * slot
            mm = t0 // 512
            lo = t0 - 512 * mm
            if lo + 256 <= 512:
                return [(mm // 4, mm % 4, lo, lo + 256, 0)]
            mm2 = mm + 1
            return [(mm // 4, mm % 4, lo, 512, 0),
                    (mm2 // 4, mm2 % 4, 0, lo + 256 - 512, 512 - lo)]

        A_tiles = {}

        def emit_slot(slot):
            at4 = slot // 4
            if at4 not in A_tiles:
                A_tiles[at4] = (psSel.tile([128, 256], F32, tag="A", name=f"A{at4}"),
                               psSel3.tile([32, 256], F32, tag="A3", name=f"A3{at4}"))
            A, A3 = A_tiles[at4]
            sl4 = slot % 4
            first = [True]

            def outA(plo, ln):
                if sl4 < 3:
                    return A[32 * sl4:32 * sl4 + 32, plo:plo + ln], 32 * sl4
                return A3[0:32, plo:plo + ln], 0

            def mmA2(plo, ln, lhsT, lbase, rhs, last=False):
                dst, ob = outA(plo, ln)
                nc.tensor.matmul(dst, lhsT, rhs, start=first[0], stop=last,
                                 skip_group_check=True, tile_position=(lbase, ob))
                first[0] = False

            if slot < 63:
                cws = sel_windows(slot)
                c4s = sel4_windows(slot)
                nmm = 4 * len(cws) + len(c4s)
                i = 0
                for b in range(4):
                    for (m, lo, hi, plo) in cws:
                        i += 1
                        mmA2(plo, hi - lo,
                             selW_sb[:, 2048 * b + 32 * slot: 2048 * b + 32 * slot + 32], 0,
                             st_tiles[(b, m)][:, lo:hi], last=(i == nmm))
                for (m, k, lo, hi, plo) in c4s:
                    i += 1
                    mmA2(plo, hi - lo,
                         selW_sb[:, 2048 * (4 + k) + 32 * slot: 2048 * (4 + k) + 32 * slot + 32], 0,
                         st4_tiles[m][:, lo:hi], last=(i == nmm))
            else:
                # head (L cols -> [:,0:128], span-0 windows saved in stHT) and
                # tail (R cols -> [:,128:256], live span-7 tiles)
                for b in range(4):
                    mmA2(0, 128,
                         selW_sb[:, 2048 * b + 32 * 63: 2048 * b + 32 * 63 + 32], 0,
                         stHT[:, 128 * b:128 * (b + 1)], last=False)
                    mmA2(128, 128,
                         selW_sb[:, 2048 * b + 32 * 63: 2048 * b + 32 * 63 + 32], 0,
                         st_tiles[(b, 7)][:, 1920:2048], last=False)
                mmA2(0, 128,
                     selW_sb[:, 2048 * 4 + 32 * 63: 2048 * 4 + 32 * 63 + 32], 0,
                     stHT[:, 512:640], last=False)
                mmA2(128, 128,
                     selW_sb[:, 2048 * 7 + 32 * 63: 2048 * 7 + 32 * 63 + 32], 0,
                     st4_tiles[7][:, 384:512], last=True)

        osc_tiles = {}

        def emit_atile_done(at4):
            """A-tile at4 complete -> AW mult + pairsum into osc_ps."""
            A, A3 = A_tiles.pop(at4)
            aw = awp.tile([128, 256], BF16, tag="aw")
            wc = Wc63_sb if at4 == 15 else Wc_sb
            nc.vector.tensor_tensor(aw[0:96, :], A[0:96, :], wc[0:96, :], OP.mult)
            nc.vector.tensor_tensor(aw[96:128, :], A3[0:32, :], wc[96:128, :], OP.mult)
            t = at4 // 2
            if t not in osc_tiles:
                osc_tiles[t] = psOsc.tile([128, 256], F32, tag="osc", name=f"osc{t}")
            nc.tensor.matmul(osc_tiles[t][64 * (at4 % 2):64 * (at4 % 2) + 64, :],
                             P_sb[:], aw[:], start=True, stop=True,
                             skip_group_check=True, tile_position=(0, 64 * (at4 % 2)))

        def emit_combine(t):
            osc = osc_tiles.pop(t)
            w0 = W0m63_sb if t == 7 else W0m_sb
            w1 = W1m63_sb if t == 7 else W1m_sb
            mixa = fin.tile([128, 256], BF16, tag="mixa")
            nc.gpsimd.tensor_scalar(mixa[:], w0[:], ovn_sb[:, 2 * t:2 * t + 1], None, OP.mult)
            mixb = fin.tile([128, 256], BF16, tag="mixb")
            nc.gpsimd.tensor_scalar(mixb[:], w1[:], ovn_sb[:, 2 * t + 1:2 * t + 2], None, OP.mult)
            mixT = fin.tile([128, 256], BF16, tag="mixT")
            nc.gpsimd.tensor_tensor(mixT[:], mixa[:], mixb[:], OP.add)
            d = fin.tile([128, 256], BF16, tag="d")
            nc.vector.tensor_tensor(d[:], osc[:], noise_sb[t][:], OP.subtract)
            mres = fin.tile([128, 256], BF16, tag="mres")
            nc.gpsimd.tensor_tensor(mres[:], d[:], mixT[:], OP.mult)
            r = fin.tile([128, 256], BF16, tag="r")
            nc.gpsimd.tensor_tensor(r[:], mres[:], noise_sb[t][:], OP.add)
            # out DMAs per slot
            for sl in range(8):
                slot = 8 * t + sl
                if slot < 63:
                    t0 = 128 + 256 * slot
                    nc.sync.dma_start(out[:, t0:t0 + 256], r[16 * sl:16 * (sl + 1), :])
                else:
                    nc.sync.dma_start(out[:, 0:128], r[112:128, 0:128])
                    nc.sync.dma_start(out[:, 16256:16384], r[112:128, 128:256])

        # drive: spans 0..7; selects trail one span behind
        stHT = cp.tile([128, 640], BF16, tag="stHT")
        emit_span(0)
        for b in range(4):
            nc.gpsimd.tensor_copy(stHT[:, 128 * b:128 * (b + 1)],
                                  st_tiles[(b, 0)][:, 0:128])
        nc.gpsimd.tensor_copy(stHT[:, 512:640], st4_tiles[0][:, 0:128])
        for m in range(1, 8):
            emit_span(m)
            for slot in range(8 * (m - 1), 8 * m):
                emit_slot(slot)
                if slot % 4 == 3:
                    emit_atile_done(slot // 4)
                if slot % 8 == 7:
                    emit_combine(slot // 8)
            # free previous-previous span tiles
            for b in range(4):
                st_tiles.pop((b, m - 2), None)
            st4_tiles.pop(m - 2, None)
        for slot in range(56, 64):
            emit_slot(slot)
            if slot % 4 == 3:
                emit_atile_done(slot // 4)
            if slot % 8 == 7:
                emit_combine(slot // 8)
    nc.finalize()
    _cache["nc"] = nc
    return nc


def _host_prep(inputs):
    st = _build_static()
    bf = ml_dtypes.bfloat16
    f0 = np.clip(np.asarray(inputs["f0"], np.float64), -0.5, 0.5)
    f0b = np.asarray(inputs["f0_baselines"], np.float64)
    erb = (0.108 * (f0b * NYQ) + 24.7) / NYQ
    f0v = np.clip(f0b + f0 * erb, 0.0, 1.0)
    f0n = MIN_F0 + f0v * F0_DIFF                                    # (B,16,64)
    hfact = np.concatenate([[1.0], np.arange(2, 2 + NH)])
    frq = (f0n[:, :, None, :] * hfact[None, None, :, None] * 0.5)   # (B,16,33,64)
    frq = frq.reshape(B, NROW, SEQ)

    coef = np.einsum("bgs,sc->bgc", frq, st["W64"])                 # (B,528,512) f64
    # logical per-chunk coef rows br = 4*hs + {A,B,C,C}; W64 col = 16m+3hs+k
    clog = np.zeros((B, NROW, 32, 16))                              # (B,g,chunk,br)
    for m in range(32):
        for hs in range(4):
            base = 16 * m + 3 * hs
            clog[:, :, m, 4 * hs + 0] = coef[:, :, base + 0]
            clog[:, :, m, 4 * hs + 1] = coef[:, :, base + 1]
            clog[:, :, m, 4 * hs + 2] = coef[:, :, base + 2]
            clog[:, :, m, 4 * hs + 3] = coef[:, :, base + 2]
    # 3-way fp16 split
    h0 = clog.astype(np.float16).astype(np.float64)
    h1 = (clog - h0).astype(np.float16).astype(np.float64)
    h2 = (clog - h0 - h1).astype(np.float16)
    splits = [h0.astype(np.float16), h1.astype(np.float16), h2]
    # coefT7[b][64*(m%2) + 4*br + s, bblk*2048 + (m//2)*128 + g]
    coefT7 = np.zeros((B, 128, 5 * 2048), np.float16)
    for b5 in range(4):
        blkg = slice(128 * b5, 128 * (b5 + 1))
        for m in range(32):
            for sp in range(3):
                rows = 64 * (m % 2) + 4 * np.arange(16) + sp
                cols = b5 * 2048 + (m // 2) * 128
                coefT7[:, rows, cols:cols + 128] = \
                    splits[sp][:, blkg, m, :].transpose(0, 2, 1)
    # block-4 span-packed: [128, 16 blocks x 64]
    coefT4s = np.zeros((B, 128, 1024), np.float16)
    for m8 in range(8):
        for kp in range(2):
            blk = 2 * m8 + kp
            for klocal in range(2):
                ch = 4 * m8 + 2 * kp + klocal
                for sp in range(3):
                    rows = 64 * klocal + 4 * np.arange(16) + sp
                    cols = 64 * blk + 32 * klocal
                    coefT4s[:, rows, cols:cols + 16] = \
                        splits[sp][:, 512:528, ch, :].transpose(0, 2, 1)

    oe = np.clip(np.asarray(inputs["osc_env"], np.float64), 0, 1)   # (B,16,64)
    he = np.clip(np.asarray(inputs["harm_env"], np.float64), 0, 1)  # (B,16,32,64)
    env_node = np.zeros((B, NROW, SEQ))
    env_node[:, 0::33, :] = oe
    for o in range(1, 33):
        env_node[:, o::33, :] = oe * he[:, :, o - 1, :]
    selWh = np.zeros((B, 128, 8 * 2048), np.float32)
    eidx = np.arange(NROW) // 33
    for b5 in range(5):
        gl = 128 if b5 < 4 else 16
        for glo in range(gl):
            g = 128 * b5 + glo
            e = eidx[g]
            for side in range(2):
                cols = 2048 * b5 + 32 * np.arange(64) + 16 * side + e
                nodes = np.minimum(np.arange(64) + side, 63)
                if side == 0:
                    nodes = np.concatenate([np.arange(63), [0]])
                else:
                    nodes = np.concatenate([np.arange(1, 64), [63]])
                selWh[:, glo, cols] = env_node[:, g, nodes]
    # b4 variants: slot-block (4+k) has env rows only at 32k..32k+16
    for k in range(1, 4):
        selWh[:, 32 * k:32 * k + 16, 2048 * (4 + k):2048 * (5 + k)] = \
            selWh[:, 0:16, 2048 * 4:2048 * 5]
    selWh = selWh.astype(bf)

    ov = np.clip(np.asarray(inputs["overall_env"], np.float64), 0, 1)  # (B,16,64)
    ovn = np.zeros((B, 128, 16), np.float32)
    for t in range(8):
        for sl in range(8):
            slot = 8 * t + sl
            p = slice(16 * sl, 16 * (sl + 1))
            if slot < 63:
                ovn[:, p, 2 * t] = ov[:, :, slot]
                ovn[:, p, 2 * t + 1] = ov[:, :, min(slot + 1, 63)]
            else:
                ovn[:, p, 2 * t] = ov[:, :, 0]
                ovn[:, p, 2 * t + 1] = ov[:, :, 63]

    nf = np.asarray(inputs["noise_frames"], np.float32)             # (B,16,64,512)
    nfT = np.ascontiguousarray(
        nf.transpose(0, 3, 2, 1).reshape(B, WS, NFR)).astype(bf)    # [ws, s*16+e]

    nstd = np.clip(np.asarray(inputs["noise_std"], np.float64), 1e-12, 1.0) * F0_DIFF
    mean_fr = f0n.transpose(0, 2, 1).reshape(B, NFR)                # fr = s*16+e
    c2_fr = -0.5 / nstd.transpose(0, 2, 1).reshape(B, NFR) ** 2
    meanb = np.zeros((B, 128, 1056), np.float32)
    c2b = np.zeros((B, 128, 1056), np.float32)
    meanb[:, :, 16:1040] = mean_fr[:, None, :]
    c2b[:, :, 16:1040] = c2_fr[:, None, :]

    per_core = []
    for b in range(B):
        per_core.append(dict(
            coefT7=coefT7[b], coefT4s=coefT4s[b], selW=selWh[b], ovn=ovn[b], nfT=nfT[b],
            meanb=meanb[b].astype(bf), c2b=c2b[b].astype(bf),
            basis64=st["basis64"],
            Cw=st["Cw"], Dc=st["Dc"], freq4=st["freq4"],
            Wc=st["Wc"], Wc63=st["Wc63"], W0m=st["W0m"], W1m=st["W1m"],
            W0m63=st["W0m63"], W1m63=st["W1m63"], P=st["P"],
        ))
    return per_core


def kernel(**inputs):
    from concourse.bass_utils import run_bass_kernel_spmd
    per_core = _host_prep(inputs)
    nc = _build_nc()
    trace = bool(os.environ.get("BASS_PROFILE"))
    res = run_bass_kernel_spmd(nc, per_core, list(range(B)), trace=trace)
    if trace and res.exec_time_ns is not None:
        print(f"HW exec time: {res.exec_time_ns} ns")
    out = np.stack([np.asarray(r["out"], np.float32) for r in res.results])
    return out


# revision 3
# speedup vs baseline: 1.0159x; 1.0159x over previous
"""Trainium2 Bass kernel v2 for nn_AudioEvent.

Per-core pipeline (batch-parallel over 8 cores):
  osc:  host-precomputed quadratic phase coeffs (per 128-sample half-segment)
        -> stage2 matmuls (f32r, exact) -> mod 1.0 (DVE/Pool) -> Sin (Act,
        big tiles, arg 2*pi*frac - pi; env weights pre-negated)
        -> env-folded select matmuls (L/R interp nodes as 32 cols) -> *W blend
        -> pairsum matmul -> osc in (slot,e)-partition layout
  noise: host-transposed frames -> windowed rDFT matmuls -> gaussian filter
        (built on-chip) -> combined irfft+overlap-add matmuls directly into
        the (slot,e) layout
  mix:  piecewise-linear via per-partition node scalars + ramp consts
  out:  bf16, per-slot DMAs, host casts to f32
"""
import os
import numpy as np
import ml_dtypes

B, NE, NH, SEQ, N, WS = 8, 16, 32, 64, 16384, 512
NYQ = 11025.0
MIN_F0 = np.float64(20.0 / NYQ)
MAX_F0 = np.float64(800.0 / NYQ)
F0_DIFF = MAX_F0 - MIN_F0
NROW = NE * 33            # 528
NFR = SEQ * NE            # 1024
NSL = 64                  # 63 interior slots + head/tail combo slot

_cache = {}


def _build_static():
    if "static" in _cache:
        return _cache["static"]
    # ---- quadratic coeffs of V = cumsum(interp weights) ----
    pos = (np.arange(N, dtype=np.float64) + 0.5) * (SEQ / N) - 0.5
    pos = np.clip(pos, 0.0, SEQ - 1)
    i0 = np.floor(pos).astype(np.int64)
    i1 = np.minimum(i0 + 1, SEQ - 1)
    w = pos - i0
    U = np.zeros((SEQ, N))
    U[i0, np.arange(N)] += 1.0 - w
    U[i1, np.arange(N)] += w
    V = np.cumsum(U, axis=1)
    W64 = np.zeros((SEQ, 512))
    for m in range(32):
        for hs in range(4):
            t0 = 512 * m + 128 * hs
            A = V[:, t0]
            C = (V[:, t0 + 2] - 2 * V[:, t0 + 1] + V[:, t0]) / 2
            Bc = V[:, t0 + 1] - V[:, t0] - C
            W64[:, 16 * m + 3 * hs + 0] = A
            W64[:, 16 * m + 3 * hs + 1] = Bc
            W64[:, 16 * m + 3 * hs + 2] = C

    # ---- stage2 basis: even/odd paired [32, 512] ----
    j = np.arange(128, dtype=np.float64)
    basis16 = np.zeros((16, 512), np.float32)
    for hs in range(4):
        basis16[3 * hs + 0, 128 * hs:128 * (hs + 1)] = 1.0
        basis16[3 * hs + 1, 128 * hs:128 * (hs + 1)] = j
        basis16[3 * hs + 2, 128 * hs:128 * (hs + 1)] = j * j
    basisE = np.zeros((128, 512), np.float32)
    basisO = np.zeros((128, 512), np.float32)
    for a in range(4):
        basisE[32 * a:32 * a + 16] = basis16
        basisO[32 * a + 16:32 * a + 32] = basis16
    # fp16 split basis: 16 logical rows -> (1, j, j2hi, j2lo) x 4 half-segs,
    # each logical row replicated at 3 split slots (4th slot zero pad);
    # row = 64*rep + 4*br + s, two 64-row replicas
    b16s = np.zeros((16, 512))
    for hs in range(4):
        sl = slice(128 * hs, 128 * (hs + 1))
        b16s[4 * hs + 0, sl] = 1.0
        b16s[4 * hs + 1, sl] = j
        j2h = np.float16(j * j).astype(np.float64)
        b16s[4 * hs + 2, sl] = j2h
        b16s[4 * hs + 3, sl] = j * j - j2h
    basis64 = np.zeros((128, 512), np.float16)
    for rep in range(2):
        for br in range(16):
            for sp in range(3):
                basis64[64 * rep + 4 * br + sp] = np.float16(b16s[br])

    # ---- DFT consts (win folded), 4 contraction groups ----
    t = np.arange(WS)
    f = np.arange(WS // 2 + 1)
    win = 0.5 - 0.5 * np.cos(2 * np.pi * t / WS)
    ang = 2 * np.pi * np.outer(t, f) / WS
    CwRe = np.cos(ang) * win[:, None]
    CwIm = -np.sin(ang) * win[:, None]
    wgt = np.full(WS // 2 + 1, 2.0)
    wgt[0] = 1.0
    wgt[-1] = 1.0
    ang2 = 2 * np.pi * np.outer(f, t) / WS
    DRe = wgt[:, None] * np.cos(ang2) / WS
    DIm = -wgt[:, None] * np.sin(ang2) / WS
    # groups: 0: re f0..127 | 1: re f128..255 | 2: im f1..128 | 3: im f129..255 + re256
    Cw = np.zeros((128, 2048))       # col = tc*512 + grp*128 + fcol
    Dc = np.zeros((128, 2048))       # col = grp*512 + tau
    freq4 = np.zeros((128, 4))
    for grp in range(4):
        if grp == 0:
            fidx, mats = np.arange(0, 128), CwRe
        elif grp == 1:
            fidx, mats = np.arange(128, 256), CwRe
        elif grp == 2:
            fidx, mats = np.arange(1, 129), CwIm
        else:
            fidx, mats = np.concatenate([np.arange(129, 256), [256]]), None
        for tc in range(4):
            trows = slice(128 * tc, 128 * (tc + 1))
            if grp < 3:
                Cw[:, tc * 512 + grp * 128: tc * 512 + grp * 128 + 128] = mats[trows][:, fidx]
            else:
                blockm = CwIm[trows][:, fidx[:-1]]
                Cw[:, tc * 512 + grp * 128: tc * 512 + grp * 128 + 127] = blockm
                Cw[:, tc * 512 + grp * 128 + 127] = CwRe[trows][:, 256]
        if grp < 3:
            Dc[:, grp * 512:(grp + 1) * 512] = (DRe if grp < 2 else DIm)[fidx]
            freq4[:, grp] = fidx / 256.0
        else:
            Dc[:127, grp * 512:(grp + 1) * 512] = DIm[fidx[:-1]]
            Dc[127, grp * 512:(grp + 1) * 512] = DRe[256]
            freq4[:127, grp] = fidx[:-1] / 256.0
            freq4[127, grp] = 1.0

    # ---- ramp consts ----
    wj = (np.arange(256) + 0.5) / 256.0
    Wc = np.zeros((128, 256))
    for p in range(128):
        Wc[p] = wj if (p // 16) % 2 == 1 else 1.0 - wj
    Wc63 = Wc.copy()
    Wc63[96:112] = np.concatenate([np.ones(128), np.zeros(128)])
    Wc63[112:128] = np.concatenate([np.zeros(128), np.ones(128)])
    W0m = np.tile(1.0 - wj, (128, 1))
    W1m = np.tile(wj, (128, 1))
    W0m63 = W0m.copy()
    W1m63 = W1m.copy()
    W0m63[112:128] = np.concatenate([np.ones(128), np.zeros(128)])
    W1m63[112:128] = np.concatenate([np.zeros(128), np.ones(128)])
    P = np.zeros((128, 64))
    for p in range(128):
        P[p, 16 * (p // 32) + p % 16] = 1.0

    bf = ml_dtypes.bfloat16
    static = dict(
        W64=W64, basis64=basis64, b16s=b16s,
        Cw=Cw.astype(bf), Dc=Dc.astype(bf), freq4=freq4.astype(np.float32),
        Wc=Wc.astype(bf), Wc63=Wc63.astype(bf),
        W0m=W0m.astype(bf), W1m=W1m.astype(bf),
        W0m63=W0m63.astype(bf), W1m63=W1m63.astype(bf),
        P=P.astype(bf),
    )
    _cache["static"] = static
    return static


def _build_nc():
    if "nc" in _cache:
        return _cache["nc"]
    from concourse import bacc
    import concourse.tile as tile
    from concourse import mybir
    from contextlib import ExitStack

    F32 = mybir.dt.float32
    F16 = mybir.dt.float16
    BF16 = mybir.dt.bfloat16
    AF = mybir.ActivationFunctionType
    OP = mybir.AluOpType
    PI = float(np.pi)

    nc = bacc.Bacc()
    # data params
    coefT7 = nc.declare_dram_parameter("coefT7", [128, 5 * 2048], F16, isOutput=False)
    selW = nc.declare_dram_parameter("selW", [128, 8 * 2048], BF16, isOutput=False)
    ovn = nc.declare_dram_parameter("ovn", [128, 16], F32, isOutput=False)
    nfT = nc.declare_dram_parameter("nfT", [512, 1024], BF16, isOutput=False)
    meanb = nc.declare_dram_parameter("meanb", [128, 1056], BF16, isOutput=False)
    c2b = nc.declare_dram_parameter("c2b", [128, 1056], BF16, isOutput=False)
    # const params
    basis64 = nc.declare_dram_parameter("basis64", [128, 512], F16, isOutput=False)
    coefT4s = nc.declare_dram_parameter("coefT4s", [128, 1024], F16, isOutput=False)
    Cw = nc.declare_dram_parameter("Cw", [128, 2048], BF16, isOutput=False)
    Dc = nc.declare_dram_parameter("Dc", [128, 2048], BF16, isOutput=False)
    freq4 = nc.declare_dram_parameter("freq4", [128, 4], F32, isOutput=False)
    Wc = nc.declare_dram_parameter("Wc", [128, 256], BF16, isOutput=False)
    Wc63 = nc.declare_dram_parameter("Wc63", [128, 256], BF16, isOutput=False)
    W0m = nc.declare_dram_parameter("W0m", [128, 256], BF16, isOutput=False)
    W1m = nc.declare_dram_parameter("W1m", [128, 256], BF16, isOutput=False)
    W0m63 = nc.declare_dram_parameter("W0m63", [128, 256], BF16, isOutput=False)
    W1m63 = nc.declare_dram_parameter("W1m63", [128, 256], BF16, isOutput=False)
    P = nc.declare_dram_parameter("P", [128, 64], BF16, isOutput=False)
    out = nc.declare_dram_parameter("out", [NE, N], BF16, isOutput=True)

    with tile.TileContext(nc) as tc, ExitStack() as ctx:
        cp = ctx.enter_context(tc.tile_pool(name="const", bufs=1))
        # noise consts first (phase A starts immediately)
        nfT_sb = [cp.tile([128, 1024], BF16, tag=f"nfT{i}", name=f"nfT{i}") for i in range(4)]
        for i in range(4):
            nc.sync.dma_start(nfT_sb[i][:], nfT[128 * i:128 * (i + 1), :])
        meanb_sb = cp.tile([128, 1056], BF16, tag="meanb")
        nc.sync.dma_start(meanb_sb[:], meanb[:])
        c2b_sb = cp.tile([128, 1056], BF16, tag="c2b")
        nc.sync.dma_start(c2b_sb[:], c2b[:])
        Cw_sb = cp.tile([128, 2048], BF16, tag="Cw")
        nc.sync.dma_start(Cw_sb[:], Cw[:])
        freq4_sb = cp.tile([128, 4], F32, tag="freq4")
        nc.sync.dma_start(freq4_sb[:], freq4[:])
        Dc_sb = cp.tile([128, 2048], BF16, tag="Dc")
        nc.sync.dma_start(Dc_sb[:], Dc[:])
        # phase B consts (queued behind; SP has slack)
        coefT7_sb = cp.tile([128, 5 * 2048], F16, tag="coefT7")
        for bb in range(5):
            nc.sync.dma_start(coefT7_sb[:, 2048 * bb:2048 * (bb + 1)],
                              coefT7[:, 2048 * bb:2048 * (bb + 1)])
        basis64_sb = cp.tile([128, 512], F16, tag="basis64")
        nc.sync.dma_start(basis64_sb[:], basis64[:])
        coefT4s_sb = cp.tile([128, 1024], F16, tag="coefT4s")
        nc.sync.dma_start(coefT4s_sb[:], coefT4s[:])
        selW_sb = cp.tile([128, 8 * 2048], BF16, tag="selW")
        for bb in range(8):
            nc.sync.dma_start(selW_sb[:, 2048 * bb:2048 * (bb + 1)],
                              selW[:, 2048 * bb:2048 * (bb + 1)])
        P_sb = cp.tile([128, 64], BF16, tag="P")
        nc.sync.dma_start(P_sb[:], P[:])
        Wc_sb = cp.tile([128, 256], BF16, tag="Wc")
        nc.sync.dma_start(Wc_sb[:], Wc[:])
        Wc63_sb = cp.tile([128, 256], BF16, tag="Wc63")
        nc.sync.dma_start(Wc63_sb[:], Wc63[:])
        W0m_sb = cp.tile([128, 256], BF16, tag="W0m")
        nc.sync.dma_start(W0m_sb[:], W0m[:])
        W1m_sb = cp.tile([128, 256], BF16, tag="W1m")
        nc.sync.dma_start(W1m_sb[:], W1m[:])
        W0m63_sb = cp.tile([128, 256], BF16, tag="W0m63")
        nc.sync.dma_start(W0m63_sb[:], W0m63[:])
        W1m63_sb = cp.tile([128, 256], BF16, tag="W1m63")
        nc.sync.dma_start(W1m63_sb[:], W1m63[:])
        ovn_sb = cp.tile([128, 16], F32, tag="ovn")
        nc.sync.dma_start(ovn_sb[:], ovn[:])
        c23b = cp.tile([128, 1], F32, tag="c23b")
        nc.vector.memset(c23b[:], float(2.0 ** 23))

        spec_sb = [cp.tile([128, 1056], BF16, tag=f"spec{g}", name=f"spec{g}") for g in range(4)]
        noise_sb = [cp.tile([128, 256], BF16, tag=f"nz{t}", name=f"nz{t}") for t in range(8)]

        # ============ Phase A: noise ============
        with ExitStack() as actx:
            fpool = actx.enter_context(tc.tile_pool(name="fp", bufs=2))
            psA = actx.enter_context(tc.tile_pool(name="psA", bufs=2, space="PSUM"))
            psNZ = actx.enter_context(tc.tile_pool(name="psNZ", bufs=2, space="PSUM"))
            for g in range(4):
                nc.gpsimd.memset(spec_sb[g][:, 0:16], 0.0)
                nc.gpsimd.memset(spec_sb[g][:, 1040:1056], 0.0)
            # build filter then immediately rfft+mult for each group
            for g in range(4):
                d = fpool.tile([128, 1056], BF16, tag="fd")
                nc.gpsimd.tensor_scalar(d[:], meanb_sb[:], freq4_sb[:, g:g + 1], None, OP.subtract)
                d2 = fpool.tile([128, 1056], BF16, tag="fd2")
                nc.gpsimd.tensor_tensor(d2[:], d[:], d[:], OP.mult)
                m2 = fpool.tile([128, 1056], BF16, tag="fm2")
                nc.gpsimd.tensor_tensor(m2[:], d2[:], c2b_sb[:], OP.mult)
                filt = fpool.tile([128, 1056], BF16, tag=f"filt{g}")
                nc.scalar.activation(filt[:], m2[:], AF.Exp)
                for h in range(2):
                    fr_sl = slice(512 * h, 512 * (h + 1))
                    sp = psA.tile([128, 512], F32, tag="rf")
                    for tcx in range(4):
                        nc.tensor.matmul(sp[:],
                                         Cw_sb[:, tcx * 512 + g * 128: tcx * 512 + g * 128 + 128],
                                         nfT_sb[tcx][:, fr_sl],
                                         start=(tcx == 0), stop=(tcx == 3))
                    srf = fpool.tile([128, 512], BF16, tag="srf")
                    nc.scalar.copy(srf[:], sp[:])
                    nc.gpsimd.tensor_tensor(spec_sb[g][:, 16 + 512 * h:16 + 512 * (h + 1)],
                                            srf[:], filt[:, 16 + 512 * h:16 + 512 * (h + 1)],
                                            OP.mult)
            # irfft + overlap-add into (slot, e) layout
            for t in range(8):
                pz = psNZ.tile([128, 256], F32, tag="nzps")
                base = 16 + 16 * (8 * t)          # spec col of fr(slot=8t)
                nslots = 7 if t == 7 else 8
                ncols = 16 * nslots
                first = True
                for g in range(4):
                    gD = Dc_sb[:, 512 * g: 512 * (g + 1)]
                    sW = spec_sb[g]
                    # A: y[fr(s), 128+j] -> [:, 0:128]
                    nc.tensor.matmul(pz[0:ncols, 0:128], sW[:, base:base + ncols],
                                     gD[:, 128:256], start=first, stop=False,
                                     skip_group_check=True)
                    first = False
                    # C: y[fr(s-1), 384+j] -> [:, 0:128]
                    nc.tensor.matmul(pz[0:ncols, 0:128], sW[:, base - 16:base - 16 + ncols],
                                     gD[:, 384:512], start=False, stop=False,
                                     skip_group_check=True)
                    # B: y[fr(s+1), j] -> [:, 128:256]
                    nc.tensor.matmul(pz[0:ncols, 128:256], sW[:, base + 16:base + 16 + ncols],
                                     gD[:, 0:128], start=False, stop=False,
                                     skip_group_check=True)
                    # D: y[fr(s), 256+j] -> [:, 128:256]
                    nc.tensor.matmul(pz[0:ncols, 128:256], sW[:, base:base + ncols],
                                     gD[:, 256:384], start=False,
                                     stop=(t < 7 and g == 3),
                                     skip_group_check=True)
                if t == 7:
                    b63 = 16 + 16 * 63
                    pz63 = psNZ.tile([32, 256], F32, tag="nz63")
                    for g in range(4):
                        gD = Dc_sb[:, 512 * g: 512 * (g + 1)]
                        sW = spec_sb[g]
                        # head: y[fr(0), j] (+ zero-pad tail term via pad cols)
                        nc.tensor.matmul(pz63[0:16, 0:128], sW[:, 16:32],
                                         gD[:, 0:128], start=(g == 0), stop=False,
                                         skip_group_check=True, tile_position=(0, 0))
                        # tail: y[fr(63), 128+j] + y[fr(62), 384+j]
                        nc.tensor.matmul(pz63[0:16, 128:256], sW[:, b63:b63 + 16],
                                         gD[:, 128:256], start=False, stop=False,
                                         skip_group_check=True, tile_position=(0, 0))
                        last = (g == 3)
                        nc.tensor.matmul(pz63[0:16, 128:256], sW[:, b63 - 16:b63],
                                         gD[:, 384:512], start=False, stop=last,
                                         skip_group_check=True, tile_position=(0, 0))
                    nc.scalar.copy(noise_sb[t][0:112, :], pz[0:112, :])
                    nz63s = fpool.tile([16, 256], BF16, tag="nz63s")
                    nc.scalar.copy(nz63s[:], pz63[0:16, :])
                    nc.sync.dma_start(noise_sb[t][112:128, :], nz63s[:])
                else:
                    nc.scalar.copy(noise_sb[t][:], pz[:])

        # ============ Phase B: oscillator bank ============
        stp = ctx.enter_context(tc.tile_pool(name="st", bufs=2))
        st4p = ctx.enter_context(tc.tile_pool(name="st4", bufs=2))
        php = ctx.enter_context(tc.tile_pool(name="phi", bufs=2))
        ph4p = ctx.enter_context(tc.tile_pool(name="phi4", bufs=2))
        awp = ctx.enter_context(tc.tile_pool(name="aw", bufs=2))
        rtp = ctx.enter_context(tc.tile_pool(name="rt", bufs=3))
        fin = ctx.enter_context(tc.tile_pool(name="fin", bufs=2))
        psPH = ctx.enter_context(tc.tile_pool(name="psPH", bufs=2, space="PSUM"))
        psP4 = ctx.enter_context(tc.tile_pool(name="psP4", bufs=1, space="PSUM"))
        psSel = ctx.enter_context(tc.tile_pool(name="psSel", bufs=2, space="PSUM"))
        psSel3 = ctx.enter_context(tc.tile_pool(name="psSel3", bufs=2, space="PSUM"))
        psOsc = ctx.enter_context(tc.tile_pool(name="psOsc", bufs=1, space="PSUM"))

        st_tiles = {}   # (b, m) -> tile ; kept for current & previous span
        st4_tiles = {}
        mod_ctr = [0]

        C23 = float(2.0 ** 23)

        def do_mod(dst_ap, src_ap):
            """dst = frac-reduced phase in [-0.5, 0.5] via +-2^23 round trick.
            Rotates across engine assignments to balance load."""
            r = mod_ctr[0] % 8
            mod_ctr[0] += 1
            n_p = src_ap.partition_size()
            n_f = src_ap.free_size()
            yt = rtp.tile([128, 512], F32, tag="yt", name="yt")
            if r < 5:
                # X: Act yt (psum+2^23 -> sbuf, rounds at f32 write)
                nc.scalar.activation(yt[0:n_p, 0:n_f], src_ap, AF.Identity,
                                     bias=c23b[0:n_p, 0:1])
            else:
                # W: DVE yt
                nc.vector.tensor_scalar(yt[0:n_p, 0:n_f], src_ap,
                                        C23, None, OP.add)
            kt = rtp.tile([128, 512], F32, tag="kt", name="kt")
            nc.gpsimd.tensor_scalar(kt[0:n_p, 0:n_f], yt[0:n_p, 0:n_f],
                                    C23, None, OP.subtract)
            nc.vector.tensor_tensor(dst_ap, src_ap, kt[0:n_p, 0:n_f], OP.subtract)

        def emit_span(m):
            """stage2 + mod + sin for span m (samples 2048m..2048m+2047)."""
            for b in range(4):
                phi = php.tile([128, 2048], F32, tag="phi", name=f"phi{b}")
                for k in range(4):
                    mm = 4 * m + k
                    p2 = 64 * (mm % 2)
                    cb = 2048 * b + (mm // 2) * 128
                    pp = psPH.tile([128, 512], F32, tag="ph")
                    nc.tensor.matmul(pp[:],
                                     coefT7_sb[p2:p2 + 64, cb:cb + 128],
                                     basis64_sb[p2:p2 + 64, :], start=True, stop=True,
                                     skip_group_check=True, tile_position=(p2, 0))
                    do_mod(phi[:, 512 * k:512 * (k + 1)], pp[:])
                st = stp.tile([128, 2048], BF16, tag=f"st{b}", name=f"st{b}_{m}")
                nc.scalar.activation(st[:], phi[:], AF.Sin, scale=2 * PI)
                st_tiles[(b, m)] = st
            # block 4 (g 512..527), 4 chunks packed at 32-aligned bases
            phi4 = ph4p.tile([128, 512], F32, tag="phi4")
            pp4 = psP4.tile([128, 512], F32, tag="ph4")
            for kp in range(2):
                nc.tensor.matmul(pp4[64 * kp:64 * kp + 64, :],
                                 coefT4s_sb[:, 64 * (2 * m + kp):64 * (2 * m + kp) + 64],
                                 basis64_sb[:], start=True, stop=True,
                                 skip_group_check=True, tile_position=(0, 64 * kp))
            do_mod(phi4[:], pp4[:])
            st4 = st4p.tile([128, 512], BF16, tag="st4")
            nc.scalar.activation(st4[:], phi4[:], AF.Sin, scale=2 * PI)
            st4_tiles[m] = st4

        def sel_windows(slot):
            """windows into st spans for slot; returns list of (m, lo, hi, psum_lo)."""
            t0 = 128 + 256 * slot
            m = t0 // 2048
            lo = t0 - 2048 * m
            if lo + 256 <= 2048:
                return [(m, lo, lo + 256, 0)]
            return [(m, lo, 2048, 0), (m + 1, 0, lo + 256 - 2048, 2048 - lo)]

        def sel4_windows(slot):
            """block4 windows: (span, chunk-in-span, lo, hi, psum_lo)."""
            t0 = 128 + 256 * slot
            mm = t0 // 512
            lo = t0 - 512 * mm
            if lo + 256 <= 512:
                return [(mm // 4, mm % 4, lo, lo + 256, 0)]
            mm2 = mm + 1
            return [(mm // 4, mm % 4, lo, 512, 0),
                    (mm2 // 4, mm2 % 4, 0, lo + 256 - 512, 512 - lo)]

        A_tiles = {}

        def emit_slot(slot):
            at4 = slot // 4
            if at4 not in A_tiles:
                A_tiles[at4] = (psSel.tile([128, 256], F32, tag="A", name=f"A{at4}"),
                               psSel3.tile([32, 256], F32, tag="A3", name=f"A3{at4}"))
            A, A3 = A_tiles[at4]
            sl4 = slot % 4
            first = [True]

            def outA(plo, ln):
                if sl4 < 3:
                    return A[32 * sl4:32 * sl4 + 32, plo:plo + ln], 32 * sl4
                return A3[0:32, plo:plo + ln], 0

            def mmA2(plo, ln, lhsT, lbase, rhs, last=False):
                dst, ob = outA(plo, ln)
                nc.tensor.matmul(dst, lhsT, rhs, start=first[0], stop=last,
                                 skip_group_check=True, tile_position=(lbase, ob))
                first[0] = False

            if slot < 63:
                cws = sel_windows(slot)
                c4s = sel4_windows(slot)
                nmm = 4 * len(cws) + len(c4s)
                i = 0
                for b in range(4):
                    for (m, lo, hi, plo) in cws:
                        i += 1
                        mmA2(plo, hi - lo,
                             selW_sb[:, 2048 * b + 32 * slot: 2048 * b + 32 * slot + 32], 0,
                             st_tiles[(b, m)][:, lo:hi], last=(i == nmm))
                for (m, k, lo, hi, plo) in c4s:
                    i += 1
                    mmA2(plo, hi - lo,
                         selW_sb[:, 2048 * (4 + k) + 32 * slot: 2048 * (4 + k) + 32 * slot + 32], 0,
                         st4_tiles[m][:, lo:hi], last=(i == nmm))
            else:
                # head (L cols -> [:,0:128], span-0 windows saved in stHT) and
                # tail (R cols -> [:,128:256], live span-7 tiles)
                for b in range(4):
                    mmA2(0, 128,
                         selW_sb[:, 2048 * b + 32 * 63: 2048 * b + 32 * 63 + 32], 0,
                         stHT[:, 128 * b:128 * (b + 1)], last=False)
                    mmA2(128, 128,
                         selW_sb[:, 2048 * b + 32 * 63: 2048 * b + 32 * 63 + 32], 0,
                         st_tiles[(b, 7)][:, 1920:2048], last=False)
                mmA2(0, 128,
                     selW_sb[:, 2048 * 4 + 32 * 63: 2048 * 4 + 32 * 63 + 32], 0,
                     stHT[:, 512:640], last=False)
                mmA2(128, 128,
                     selW_sb[:, 2048 * 7 + 32 * 63: 2048 * 7 + 32 * 63 + 32], 0,
                     st4_tiles[7][:, 384:512], last=True)

        osc_tiles = {}

        def emit_atile_done(at4):
            """A-tile at4 complete -> AW mult + pairsum into osc_ps."""
            A, A3 = A_tiles.pop(at4)
            aw = awp.tile([128, 256], BF16, tag="aw")
            wc = Wc63_sb if at4 == 15 else Wc_sb
            nc.vector.tensor_tensor(aw[0:96, :], A[0:96, :], wc[0:96, :], OP.mult)
            nc.vector.tensor_tensor(aw[96:128, :], A3[0:32, :], wc[96:128, :], OP.mult)
            t = at4 // 2
            if t not in osc_tiles:
                osc_tiles[t] = psOsc.tile([128, 256], F32, tag="osc", name=f"osc{t}")
            nc.tensor.matmul(osc_tiles[t][64 * (at4 % 2):64 * (at4 % 2) + 64, :],
                             P_sb[:], aw[:], start=True, stop=True,
                             skip_group_check=True, tile_position=(0, 64 * (at4 % 2)))

        def emit_combine(t):
            osc = osc_tiles.pop(t)
            w0 = W0m63_sb if t == 7 else W0m_sb
            w1 = W1m63_sb if t == 7 else W1m_sb
            mixa = fin.tile([128, 256], BF16, tag="mixa")
            nc.gpsimd.tensor_scalar(mixa[:], w0[:], ovn_sb[:, 2 * t:2 * t + 1], None, OP.mult)
            mixb = fin.tile([128, 256], BF16, tag="mixb")
            nc.gpsimd.tensor_scalar(mixb[:], w1[:], ovn_sb[:, 2 * t + 1:2 * t + 2], None, OP.mult)
            mixT = fin.tile([128, 256], BF16, tag="mixT")
            nc.gpsimd.tensor_tensor(mixT[:], mixa[:], mixb[:], OP.add)
            d = fin.tile([128, 256], BF16, tag="d")
            nc.vector.tensor_tensor(d[:], osc[:], noise_sb[t][:], OP.subtract)
            mres = fin.tile([128, 256], BF16, tag="mres")
            nc.gpsimd.tensor_tensor(mres[:], d[:], mixT[:], OP.mult)
            r = fin.tile([128, 256], BF16, tag="r")
            nc.gpsimd.tensor_tensor(r[:], mres[:], noise_sb[t][:], OP.add)
            # out DMAs per slot
            for sl in range(8):
                slot = 8 * t + sl
                if slot < 63:
                    t0 = 128 + 256 * slot
                    nc.sync.dma_start(out[:, t0:t0 + 256], r[16 * sl:16 * (sl + 1), :])
                else:
                    nc.sync.dma_start(out[:, 0:128], r[112:128, 0:128])
                    nc.sync.dma_start(out[:, 16256:16384], r[112:128, 128:256])

        # drive: spans 0..7; selects trail one span behind
        stHT = cp.tile([128, 640], BF16, tag="stHT")
        emit_span(0)
        for b in range(4):
            nc.gpsimd.tensor_copy(stHT[:, 128 * b:128 * (b + 1)],
                                  st_tiles[(b, 0)][:, 0:128])
        nc.gpsimd.tensor_copy(stHT[:, 512:640], st4_tiles[0][:, 0:128])
        for m in range(1, 8):
            emit_span(m)
            for slot in range(8 * (m - 1), 8 * m):
                emit_slot(slot)
                if slot % 4 == 3:
                    emit_atile_done(slot // 4)
                if slot % 8 == 7:
                    emit_combine(slot // 8)
            # free previous-previous span tiles
            for b in range(4):
                st_tiles.pop((b, m - 2), None)
            st4_tiles.pop(m - 2, None)
        for slot in range(56, 64):
            emit_slot(slot)
            if slot % 4 == 3:
                emit_atile_done(slot // 4)
            if slot % 8 == 7:
                emit_combine(slot // 8)
    nc.finalize()
    _cache["nc"] = nc
    return nc


def _host_prep(inputs):
    st = _build_static()
    bf = ml_dtypes.bfloat16
    f0 = np.clip(np.asarray(inputs["f0"], np.float64), -0.5, 0.5)
    f0b = np.asarray(inputs["f0_baselines"], np.float64)
    erb = (0.108 * (f0b * NYQ) + 24.7) / NYQ
    f0v = np.clip(f0b + f0 * erb, 0.0, 1.0)
    f0n = MIN_F0 + f0v * F0_DIFF                                    # (B,16,64)
    hfact = np.concatenate([[1.0], np.arange(2, 2 + NH)])
    frq = (f0n[:, :, None, :] * hfact[None, None, :, None] * 0.5)   # (B,16,33,64)
    frq = frq.reshape(B, NROW, SEQ)

    coef = np.einsum("bgs,sc->bgc", frq, st["W64"])                 # (B,528,512) f64
    # logical per-chunk coef rows br = 4*hs + {A,B,C,C}; W64 col = 16m+3hs+k
    clog = np.zeros((B, NROW, 32, 16))                              # (B,g,chunk,br)
    for m in range(32):
        for hs in range(4):
            base = 16 * m + 3 * hs
            clog[:, :, m, 4 * hs + 0] = coef[:, :, base + 0]
            clog[:, :, m, 4 * hs + 1] = coef[:, :, base + 1]
            clog[:, :, m, 4 * hs + 2] = coef[:, :, base + 2]
            clog[:, :, m, 4 * hs + 3] = coef[:, :, base + 2]
    # 3-way fp16 split
    h0 = clog.astype(np.float16).astype(np.float64)
    h1 = (clog - h0).astype(np.float16).astype(np.float64)
    h2 = (clog - h0 - h1).astype(np.float16)
    splits = [h0.astype(np.float16), h1.astype(np.float16), h2]
    # coefT7[b][64*(m%2) + 4*br + s, bblk*2048 + (m//2)*128 + g]
    coefT7 = np.zeros((B, 128, 5 * 2048), np.float16)
    for b5 in range(4):
        blkg = slice(128 * b5, 128 * (b5 + 1))
        for m in range(32):
            for sp in range(3):
                rows = 64 * (m % 2) + 4 * np.arange(16) + sp
                cols = b5 * 2048 + (m // 2) * 128
                coefT7[:, rows, cols:cols + 128] = \
                    splits[sp][:, blkg, m, :].transpose(0, 2, 1)
    # block-4 span-packed: [128, 16 blocks x 64]
    coefT4s = np.zeros((B, 128, 1024), np.float16)
    for m8 in range(8):
        for kp in range(2):
            blk = 2 * m8 + kp
            for klocal in range(2):
                ch = 4 * m8 + 2 * kp + klocal
                for sp in range(3):
                    rows = 64 * klocal + 4 * np.arange(16) + sp
                    cols = 64 * blk + 32 * klocal
                    coefT4s[:, rows, cols:cols + 16] = \
                        splits[sp][:, 512:528, ch, :].transpose(0, 2, 1)

    oe = np.clip(np.asarray(inputs["osc_env"], np.float64), 0, 1)   # (B,16,64)
    he = np.clip(np.asarray(inputs["harm_env"], np.float64), 0, 1)  # (B,16,32,64)
    env_node = np.zeros((B, NROW, SEQ))
    env_node[:, 0::33, :] = oe
    for o in range(1, 33):
        env_node[:, o::33, :] = oe * he[:, :, o - 1, :]
    selWh = np.zeros((B, 128, 8 * 2048), np.float32)
    eidx = np.arange(NROW) // 33
    for b5 in range(5):
        gl = 128 if b5 < 4 else 16
        for glo in range(gl):
            g = 128 * b5 + glo
            e = eidx[g]
            for side in range(2):
                cols = 2048 * b5 + 32 * np.arange(64) + 16 * side + e
                nodes = np.minimum(np.arange(64) + side, 63)
                if side == 0:
                    nodes = np.concatenate([np.arange(63), [0]])
                else:
                    nodes = np.concatenate([np.arange(1, 64), [63]])
                selWh[:, glo, cols] = env_node[:, g, nodes]
    # b4 variants: slot-block (4+k) has env rows only at 32k..32k+16
    for k in range(1, 4):
        selWh[:, 32 * k:32 * k + 16, 2048 * (4 + k):2048 * (5 + k)] = \
            selWh[:, 0:16, 2048 * 4:2048 * 5]
    selWh = selWh.astype(bf)

    ov = np.clip(np.asarray(inputs["overall_env"], np.float64), 0, 1)  # (B,16,64)
    ovn = np.zeros((B, 128, 16), np.float32)
    for t in range(8):
        for sl in range(8):
            slot = 8 * t + sl
            p = slice(16 * sl, 16 * (sl + 1))
            if slot < 63:
                ovn[:, p, 2 * t] = ov[:, :, slot]
                ovn[:, p, 2 * t + 1] = ov[:, :, min(slot + 1, 63)]
            else:
                ovn[:, p, 2 * t] = ov[:, :, 0]
                ovn[:, p, 2 * t + 1] = ov[:, :, 63]

    nf = np.asarray(inputs["noise_frames"], np.float32)             # (B,16,64,512)
    nfT = np.ascontiguousarray(
        nf.transpose(0, 3, 2, 1).reshape(B, WS, NFR)).astype(bf)    # [ws, s*16+e]

    nstd = np.clip(np.asarray(inputs["noise_std"], np.float64), 1e-12, 1.0) * F0_DIFF
    mean_fr = f0n.transpose(0, 2, 1).reshape(B, NFR)                # fr = s*16+e
    c2_fr = -0.5 / nstd.transpose(0, 2, 1).reshape(B, NFR) ** 2
    meanb = np.zeros((B, 128, 1056), np.float32)
    c2b = np.zeros((B, 128, 1056), np.float32)
    meanb[:, :, 16:1040] = mean_fr[:, None, :]
    c2b[:, :, 16:1040] = c2_fr[:, None, :]

    per_core = []
    for b in range(B):
        per_core.append(dict(
            coefT7=coefT7[b], coefT4s=coefT4s[b], selW=selWh[b], ovn=ovn[b], nfT=nfT[b],
            meanb=meanb[b].astype(bf), c2b=c2b[b].astype(bf),
            basis64=st["basis64"],
            Cw=st["Cw"], Dc=st["Dc"], freq4=st["freq4"],
            Wc=st["Wc"], Wc63=st["Wc63"], W0m=st["W0m"], W1m=st["W1m"],
            W0m63=st["W0m63"], W1m63=st["W1m63"], P=st["P"],
        ))
    return per_core


def kernel(**inputs):
    from concourse.bass_utils import run_bass_kernel_spmd
    per_core = _host_prep(inputs)
    nc = _build_nc()
    trace = bool(os.environ.get("BASS_PROFILE"))
    res = run_bass_kernel_spmd(nc, per_core, list(range(B)), trace=trace)
    if trace and res.exec_time_ns is not None:
        print(f"HW exec time: {res.exec_time_ns} ns")
    out = np.stack([np.asarray(r["out"], np.float32) for r in res.results])
    return out
